# revision 43
# baseline (speedup 1.0000x reference)
"""QRNN fo-pooling kernel for Trainium2 (Bass/Tile), batch-sharded across 8 cores.

Reference computation (per (b, h) element, sequential over t):
    F, Z, O = split(Y, 3, axis=2); F = sigmoid(F); Z = tanh(Z); O = sigmoid(O)
    c_t = F_t * c_{t-1} + (1 - F_t) * Z_t
    h_t = O_t * c_t
    out = concat([init_h, h], axis=0)

Final design (HW-A/B-tuned on TRN2), per batch b and t-chunk j:
  - F/Z slab loads on the SP HWDGE ring ([128, 1024] tiles, 4KB contiguous
    per partition vs the 512B runs of per-gate h-block loads); O loads ride
    SWDGE (gpsimd) for extra DMA-ring parallelism (-5us measured)
  - PE transposes 128x128 chunks into ps_f (bufs=1) / ps_z (bufs=2) PSUM
    tiles; Z gets the double buffer because ACT drains F first (-9us)
  - ACT (N=1024 activations, PSUM-sourced): s_neg = sigmoid(-F) = 1-f,
    zt = tanh(Z), and sigmoid(O) early so phase 3 never waits on ACT.
    Stores must NOT ride the ACT or SP queues (store_sp cost +18us)
  - Pool computes f = 1 - s_neg, DVE computes zf = s_neg * zt into full-b
    tensors [128, HB, TJ, 128]
  - the recurrence runs as tensor_tensor_scan per (b, hb) in N=256 pair
    chunks chained via initial=c[..., -1:]. The scan ALU feedback runs at
    ~2 cycles/element (measured 300ns + 1.98ns*N per instruction), so
    N=128 chunking costs 84us of DVE (was the hidden pacer) while N=256/512
    batching costs 42-54us; pairs balance DVE busy vs pipeline tail
  - phase 3 per (b, j): PE transposes c back to natural layout in [128,512]
    halves, DVE multiplies with sigmoid(O), stores go out through SWDGE
    with 2KB contiguous rows, keeping the HWDGE rings free for loads
Local (contended) HW: ~128us/rep vs ~110us pure-DMA floor (33.6MB/core);
baseline measured ~139us under matched conditions.
"""

import numpy as np

import concourse.bacc as bacc
import concourse.bass as bass
import concourse.mybir as mybir
import concourse.tile as tile
from concourse.bass_utils import run_bass_kernel_spmd
from concourse.masks import make_identity


T, B, H = 512, 32, 1024
N_CORES = 8
BS = B // N_CORES  # batches per core
P = 128
HB = H // P  # h-blocks
TJ = T // P  # t-chunks

FP32 = mybir.dt.float32

_nc_cache = []


def _build_bass(
    repeat: int = 1,
    dma_only: bool = False,
    staggered: bool = False,
    defer: int = 0,
    fz_combined: bool = False,
    p3full: bool = False,
    o_swdge: bool = True,
    z_swdge: bool = False,
    store_sp: bool = False,
    psum_swap: bool = True,
    scan_fake: bool = False,
    scan_batch: int = 4,
    scan_mode: str = "pairs",  # "batch" | "pairs" | "hybrid"
) -> bass.Bass:
    nc = bacc.Bacc("TRN2", target_bir_lowering=False)
    y = nc.declare_dram_parameter("Y", [T, BS, 3 * H], FP32, isOutput=False)
    init_c = nc.declare_dram_parameter("init_c", [1, BS, H], FP32, isOutput=False)
    init_h = nc.declare_dram_parameter("init_h", [1, BS, H], FP32, isOutput=False)
    out = nc.declare_dram_parameter("out", [T + 1, BS, H], FP32, isOutput=True)

    with tile.TileContext(nc) as tc:
        with (
            tc.tile_pool(name="sb", bufs=3) as sb,
            tc.tile_pool(name="psum", bufs=2, space="PSUM") as psum,
            tc.tile_pool(name="singles", bufs=1) as singles,
        ):
            ident = singles.tile([P, P], FP32)
            make_identity(nc, ident)

            # out[0] = init_h[0] (row 0 of the output is the initial h)
            nc.sync.dma_start(out=out[0, :, :], in_=init_h[0, :, :])

            # [t, b, c] -> [p, j, b, c] with t = j*128 + p
            yr = y[:, :, :].rearrange("(j p) b c -> p j b c", p=P)
            outr = out[1 : T + 1, :, :].rearrange("(j p) b h -> p j b h", p=P)
            # all initial states in one load: [p=h%128, b, hb]
            ic_all = singles.tile([P, BS, HB], FP32)
            nc.sync.dma_start(
                out=ic_all,
                in_=init_c[0, :, :].rearrange("b (hb p) -> p b hb", p=P),
            )

            def _dma_only_body():
                # measurement probe: identical DMA traffic/queue structure to
                # the real kernel, no compute
                zero = singles.tile([P, H], FP32, tag="zero")
                nc.vector.memset(zero, 0.0)
                for b in range(BS):
                    for j in range(TJ):
                        fsl = sb.tile([P, H], FP32, tag="fsl", bufs=6)
                        nc.sync.dma_start(out=fsl, in_=yr[:, j, b, 0:H])
                        zsl = sb.tile([P, H], FP32, tag="zsl", bufs=6)
                        nc.sync.dma_start(out=zsl, in_=yr[:, j, b, H : 2 * H])
                        osl = sb.tile([P, H], FP32, tag="osl", bufs=6)
                        nc.sync.dma_start(out=osl, in_=yr[:, j, b, 2 * H : 3 * H])
                        for half in range(2):
                            nc.gpsimd.dma_start(
                                out=outr[
                                    :, j, b,
                                    half * (H // 2) : (half + 1) * (H // 2),
                                ],
                                in_=zero[:, half * (H // 2) : (half + 1) * (H // 2)],
                            )

            def _phase3(b, j, c_t, o_sig):
                # c back to natural layout, h = sigmoid(O)*c, store
                nchunk = 1 if p3full else 2
                w = H // nchunk
                for half in range(nchunk):
                    ps_c = psum.tile([P, w], FP32, tag="ps_c", bufs=2)
                    for hh in range(HB // nchunk):
                        hb = half * (HB // nchunk) + hh
                        nc.tensor.transpose(
                            ps_c[:, hh * P : (hh + 1) * P],
                            c_t[:, hb, j, :],
                            ident,
                        )
                    h_out = sb.tile([P, w], FP32, tag="h_out", bufs=4 if p3full else 8)
                    nc.vector.tensor_mul(
                        h_out,
                        o_sig[:, half * w : (half + 1) * w],
                        ps_c,
                    )
                    # store via SWDGE: 2-4KB contiguous rows, keeps the
                    # HWDGE rings free for loads
                    st_eng = nc.sync if store_sp else nc.gpsimd
                    st_eng.dma_start(
                        out=outr[:, j, b, half * w : (half + 1) * w],
                        in_=h_out,
                    )

            def _rep_body():
                if dma_only:
                    _dma_only_body()
                    return
                # phase 3 runs one chunk behind phase 1/2 so the PE never
                # head-of-line blocks on scans between transpose groups
                pending = []
                ldb = 6 if scan_batch == 1 else 4
                for b in range(BS):
                    # c keeps full-b layout [p=h%128, hb, j, t%128] (phase 3
                    # and the chunk-chain initial read it across j)
                    c_t = sb.tile([P, HB, TJ, P], FP32, tag="c_t", bufs=2)
                    if scan_batch > 1:
                        f_tb = sb.tile([P, HB, TJ, P], FP32, tag="f_tb", bufs=2)
                        zfb = sb.tile([P, HB, TJ, P], FP32, tag="zfb", bufs=2)

                    for j in range(TJ):
                        # phase 1: load F/Z slabs (4KB contiguous rows),
                        # prefetch O, transpose, activations. Deep load
                        # buffering so the DMA queue never stalls on tiles.
                        if fz_combined:
                            fzsl = sb.tile([P, 2 * H], FP32, tag="fzsl", bufs=3)
                            nc.sync.dma_start(out=fzsl, in_=yr[:, j, b, 0 : 2 * H])
                            fsl = fzsl[:, 0:H]
                            zsl = fzsl[:, H : 2 * H]
                        else:
                            fsl = sb.tile([P, H], FP32, tag="fsl", bufs=ldb)
                            nc.sync.dma_start(out=fsl, in_=yr[:, j, b, 0:H])
                            zsl = sb.tile([P, H], FP32, tag="zsl", bufs=ldb)
                            z_eng = nc.gpsimd if z_swdge else nc.sync
                            z_eng.dma_start(out=zsl, in_=yr[:, j, b, H : 2 * H])
                        osl = sb.tile([P, H], FP32, tag="osl", bufs=ldb)
                        o_eng = nc.gpsimd if o_swdge else nc.sync
                        o_eng.dma_start(out=osl, in_=yr[:, j, b, 2 * H : 3 * H])

                        fb, zb = (1, 2) if psum_swap else (2, 1)
                        ps_f = psum.tile([P, H], FP32, tag="ps_f", bufs=1 if p3full else fb)
                        ps_z = psum.tile([P, H], FP32, tag="ps_z", bufs=zb)
                        for hb in range(HB):
                            nc.tensor.transpose(
                                ps_f[:, hb * P : (hb + 1) * P],
                                fsl[:, hb * P : (hb + 1) * P],
                                ident,
                            )
                        for hb in range(HB):
                            nc.tensor.transpose(
                                ps_z[:, hb * P : (hb + 1) * P],
                                zsl[:, hb * P : (hb + 1) * P],
                                ident,
                            )

                        # ACT drains PSUM: s_neg = 1 - sigmoid(F), zt = tanh(Z)
                        s_neg = sb.tile([P, H], FP32, tag="s_neg", bufs=3)
                        nc.scalar.activation(
                            s_neg, ps_f, mybir.ActivationFunctionType.Sigmoid,
                            scale=-1.0,
                        )
                        zt = sb.tile([P, H], FP32, tag="zt", bufs=3)
                        nc.scalar.activation(
                            zt, ps_z, mybir.ActivationFunctionType.Tanh
                        )
                        # sigmoid(O) early so phase 3 never waits on ACT
                        o_sig = sb.tile([P, H], FP32, tag="o_sig", bufs=ldb)
                        nc.scalar.activation(
                            o_sig, osl, mybir.ActivationFunctionType.Sigmoid
                        )

                        sr = s_neg.rearrange("p (hb t) -> p hb t", hb=HB)
                        zr = zt.rearrange("p (hb t) -> p hb t", hb=HB)
                        if scan_batch > 1:
                            f_t = f_tb[:, :, j, :]
                            zf = zfb[:, :, j, :]
                        else:
                            # per-j gate tensors [p, hb, t] — consumed by this
                            # j's scans immediately, no persistence needed
                            f_t = sb.tile([P, HB, P], FP32, tag="f_t", bufs=3)
                            zf = sb.tile([P, HB, P], FP32, tag="zf", bufs=3)
                        # f = 1 - s_neg on Pool; zf = s_neg * tanh(z) on DVE
                        nc.gpsimd.tensor_scalar(
                            f_t, sr, -1.0, 1.0,
                            op0=mybir.AluOpType.mult, op1=mybir.AluOpType.add,
                        )
                        nc.vector.tensor_mul(zf, zr, sr)

                        if scan_batch > 1:
                            # scans emitted at pair/batch boundaries below
                            pending.append((b, j, c_t, o_sig))
                            pairs = scan_mode == "pairs" or (
                                scan_mode == "hybrid" and b == BS - 1
                            )
                            if pairs and j % 2 == 1:
                                j0 = j - 1
                                for hb in range(HB):
                                    nc.vector.tensor_tensor_scan(
                                        c_t[:, hb, j0 : j + 1, :].rearrange(
                                            "p j t -> p (j t)"
                                        ),
                                        f_tb[:, hb, j0 : j + 1, :].rearrange(
                                            "p j t -> p (j t)"
                                        ),
                                        zfb[:, hb, j0 : j + 1, :].rearrange(
                                            "p j t -> p (j t)"
                                        ),
                                        initial=(
                                            ic_all[:, b, hb : hb + 1]
                                            if j0 == 0
                                            else c_t[:, hb, j0 - 1, P - 1 : P]
                                        ),
                                        op0=mybir.AluOpType.mult,
                                        op1=mybir.AluOpType.add,
                                    )
                                while pending:
                                    _phase3(*pending.pop(0))
                            continue

                        # phase 2: chained chunk scans — c for this t-chunk is
                        # ready as soon as this chunk's gates are, instead of
                        # waiting for the whole sequence
                        if scan_fake:
                            # timing diagnostic only: same shapes/deps, no scan
                            nc.vector.tensor_mul(
                                c_t[:, :, j, :],
                                f_t.rearrange("p hb t -> p (hb t)").rearrange(
                                    "p (hb t) -> p hb t", hb=HB
                                ),
                                zf,
                            )
                        else:
                            for hb in range(HB):
                                nc.vector.tensor_tensor_scan(
                                    c_t[:, hb, j, :],
                                    f_t[:, hb, :],
                                    zf[:, hb, :],
                                    initial=(
                                        ic_all[:, b, hb : hb + 1]
                                        if j == 0
                                        else c_t[:, hb, j - 1, P - 1 : P]
                                    ),
                                    op0=mybir.AluOpType.mult,
                                    op1=mybir.AluOpType.add,
                                )

                        pending.append((b, j, c_t, o_sig))
                        if len(pending) > defer:
                            _phase3(*pending.pop(0))

                    if scan_batch > 1 and not (
                        scan_mode == "pairs"
                        or (scan_mode == "hybrid" and b == BS - 1)
                    ):
                        # batched recurrence: one N=512 scan per (b, hb)
                        for hb in range(HB):
                            nc.vector.tensor_tensor_scan(
                                c_t[:, hb, :, :].rearrange("p j t -> p (j t)"),
                                f_tb[:, hb, :, :].rearrange("p j t -> p (j t)"),
                                zfb[:, hb, :, :].rearrange("p j t -> p (j t)"),
                                initial=ic_all[:, b, hb : hb + 1],
                                op0=mybir.AluOpType.mult,
                                op1=mybir.AluOpType.add,
                            )
                        while pending:
                            _phase3(*pending.pop(0))

                for args in pending:
                    _phase3(*args)

            if repeat == 1:
                _rep_body()
            else:
                # timing mode: hardware loop keeps the NEFF size constant in
                # `repeat`, so two loop bounds can be wall-clock diffed
                with tc.For_i(0, repeat, 1, staggered_reset=staggered):
                    _rep_body()
    nc.compile()
    return nc


def _get_nc() -> bass.Bass:
    if not _nc_cache:
        _nc_cache.append(_build_bass())
    return _nc_cache[0]


def kernel(Y: np.ndarray, init_c: np.ndarray, init_h: np.ndarray) -> np.ndarray:
    Y = np.ascontiguousarray(np.asarray(Y, dtype=np.float32))
    init_c = np.ascontiguousarray(np.asarray(init_c, dtype=np.float32))
    init_h = np.ascontiguousarray(np.asarray(init_h, dtype=np.float32))

    in_maps = []
    for k in range(N_CORES):
        sl = slice(k * BS, (k + 1) * BS)
        in_maps.append(
            {
                "Y": np.ascontiguousarray(Y[:, sl, :]),
                "init_c": np.ascontiguousarray(init_c[:, sl, :]),
                "init_h": np.ascontiguousarray(init_h[:, sl, :]),
            }
        )

    nc = _get_nc()
    res = run_bass_kernel_spmd(nc, in_maps, core_ids=list(range(N_CORES)))
    return np.concatenate([r["out"] for r in res.results], axis=1)


# revision 48
# speedup vs baseline: 1.0137x; 1.0137x over previous
"""QRNN fo-pooling kernel for Trainium2 (Bass/Tile), batch-sharded across 8 cores.

Reference computation (per (b, h) element, sequential over t):
    F, Z, O = split(Y, 3, axis=2); F = sigmoid(F); Z = tanh(Z); O = sigmoid(O)
    c_t = F_t * c_{t-1} + (1 - F_t) * Z_t
    h_t = O_t * c_t
    out = concat([init_h, h], axis=0)

Final design (HW-A/B-tuned on TRN2), per batch b and t-chunk j:
  - F/Z slab loads on the SP HWDGE ring ([128, 1024] tiles, 4KB contiguous
    per partition vs the 512B runs of per-gate h-block loads); O loads ride
    SWDGE (gpsimd) for extra DMA-ring parallelism (-5us measured)
  - PE transposes 128x128 chunks into ps_f (bufs=1) / ps_z (bufs=2) PSUM
    tiles; Z gets the double buffer because ACT drains F first (-9us)
  - ACT (N=1024 activations, PSUM-sourced): s_neg = sigmoid(-F) = 1-f,
    zt = tanh(Z), and sigmoid(O) early so phase 3 never waits on ACT.
    Stores must NOT ride the ACT or SP queues (store_sp cost +18us)
  - Pool computes f = 1 - s_neg, DVE computes zf = s_neg * zt into full-b
    tensors [128, HB, TJ, 128]
  - the recurrence runs as tensor_tensor_scan per (b, hb) in N=256 pair
    chunks chained via initial=c[..., -1:]. The scan ALU feedback runs at
    ~2 cycles/element (measured 300ns + 1.98ns*N per instruction), so
    N=128 chunking costs 84us of DVE (was the hidden pacer) while N=256/512
    batching costs 42-54us; pairs balance DVE busy vs pipeline tail
  - phase 3 per (b, j): PE transposes c back to natural layout in [128,512]
    halves, DVE multiplies with sigmoid(O), stores go out through SWDGE
    with 2KB contiguous rows, keeping the HWDGE rings free for loads
Local (contended) HW: ~128us/rep vs ~110us pure-DMA floor (33.6MB/core);
baseline measured ~139us under matched conditions.
"""

import numpy as np

import concourse.bacc as bacc
import concourse.bass as bass
import concourse.mybir as mybir
import concourse.tile as tile
from concourse.bass_utils import run_bass_kernel_spmd
from concourse.masks import make_identity


T, B, H = 512, 32, 1024
N_CORES = 8
BS = B // N_CORES  # batches per core
P = 128
HB = H // P  # h-blocks
TJ = T // P  # t-chunks

FP32 = mybir.dt.float32

_nc_cache = []


def _build_bass(
    repeat: int = 1,
    dma_only: bool = False,
    staggered: bool = False,
    defer: int = 0,
    fz_combined: bool = False,
    p3full: bool = False,
    o_swdge: bool = True,
    z_swdge: bool = False,
    store_sp: bool = False,
    psum_swap: bool = True,
    scan_fake: bool = False,
    scan_batch: int = 4,
    scan_mode: str = "pairs",  # "batch" | "pairs" | "hybrid"
    zf_split: bool = False,
    p3big: bool = False,
) -> bass.Bass:
    nc = bacc.Bacc("TRN2", target_bir_lowering=False)
    y = nc.declare_dram_parameter("Y", [T, BS, 3 * H], FP32, isOutput=False)
    init_c = nc.declare_dram_parameter("init_c", [1, BS, H], FP32, isOutput=False)
    init_h = nc.declare_dram_parameter("init_h", [1, BS, H], FP32, isOutput=False)
    out = nc.declare_dram_parameter("out", [T + 1, BS, H], FP32, isOutput=True)

    with tile.TileContext(nc) as tc:
        with (
            tc.tile_pool(name="sb", bufs=3) as sb,
            tc.tile_pool(name="psum", bufs=2, space="PSUM") as psum,
            tc.tile_pool(name="singles", bufs=1) as singles,
        ):
            ident = singles.tile([P, P], FP32)
            make_identity(nc, ident)

            # out[0] = init_h[0] (row 0 of the output is the initial h)
            nc.sync.dma_start(out=out[0, :, :], in_=init_h[0, :, :])

            # [t, b, c] -> [p, j, b, c] with t = j*128 + p
            yr = y[:, :, :].rearrange("(j p) b c -> p j b c", p=P)
            outr = out[1 : T + 1, :, :].rearrange("(j p) b h -> p j b h", p=P)
            # all initial states in one load: [p=h%128, b, hb]
            ic_all = singles.tile([P, BS, HB], FP32)
            nc.sync.dma_start(
                out=ic_all,
                in_=init_c[0, :, :].rearrange("b (hb p) -> p b hb", p=P),
            )

            def _dma_only_body():
                # measurement probe: identical DMA traffic/queue structure to
                # the real kernel, no compute
                zero = singles.tile([P, H], FP32, tag="zero")
                nc.vector.memset(zero, 0.0)
                for b in range(BS):
                    for j in range(TJ):
                        fsl = sb.tile([P, H], FP32, tag="fsl", bufs=6)
                        nc.sync.dma_start(out=fsl, in_=yr[:, j, b, 0:H])
                        zsl = sb.tile([P, H], FP32, tag="zsl", bufs=6)
                        nc.sync.dma_start(out=zsl, in_=yr[:, j, b, H : 2 * H])
                        osl = sb.tile([P, H], FP32, tag="osl", bufs=6)
                        nc.sync.dma_start(out=osl, in_=yr[:, j, b, 2 * H : 3 * H])
                        for half in range(2):
                            nc.gpsimd.dma_start(
                                out=outr[
                                    :, j, b,
                                    half * (H // 2) : (half + 1) * (H // 2),
                                ],
                                in_=zero[:, half * (H // 2) : (half + 1) * (H // 2)],
                            )

            def _phase3(b, j, c_t, o_sig):
                # c back to natural layout, h = sigmoid(O)*c, store
                nchunk = 1 if (p3full or p3big) else 2
                w = H // nchunk
                for half in range(nchunk):
                    ps_c = psum.tile([P, w], FP32, tag="ps_c", bufs=1 if p3big else 2)
                    for hh in range(HB // nchunk):
                        hb = half * (HB // nchunk) + hh
                        nc.tensor.transpose(
                            ps_c[:, hh * P : (hh + 1) * P],
                            c_t[:, hb, j, :],
                            ident,
                        )
                    h_out = sb.tile([P, w], FP32, tag="h_out", bufs=4 if (p3full or p3big) else 8)
                    nc.vector.tensor_mul(
                        h_out,
                        o_sig[:, half * w : (half + 1) * w],
                        ps_c,
                    )
                    # store via SWDGE: 2-4KB contiguous rows, keeps the
                    # HWDGE rings free for loads
                    st_eng = nc.sync if store_sp else nc.gpsimd
                    st_eng.dma_start(
                        out=outr[:, j, b, half * w : (half + 1) * w],
                        in_=h_out,
                    )

            def _rep_body():
                if dma_only:
                    _dma_only_body()
                    return
                # phase 3 runs one chunk behind phase 1/2 so the PE never
                # head-of-line blocks on scans between transpose groups
                pending = []
                ldb = 6 if scan_batch == 1 else 4
                for b in range(BS):
                    # c keeps full-b layout [p=h%128, hb, j, t%128] (phase 3
                    # and the chunk-chain initial read it across j)
                    c_t = sb.tile([P, HB, TJ, P], FP32, tag="c_t", bufs=2)
                    if scan_batch > 1:
                        f_tb = sb.tile([P, HB, TJ, P], FP32, tag="f_tb", bufs=2)
                        zfb = sb.tile([P, HB, TJ, P], FP32, tag="zfb", bufs=2)

                    for j in range(TJ):
                        # phase 1: load F/Z slabs (4KB contiguous rows),
                        # prefetch O, transpose, activations. Deep load
                        # buffering so the DMA queue never stalls on tiles.
                        if fz_combined:
                            fzsl = sb.tile([P, 2 * H], FP32, tag="fzsl", bufs=3)
                            nc.sync.dma_start(out=fzsl, in_=yr[:, j, b, 0 : 2 * H])
                            fsl = fzsl[:, 0:H]
                            zsl = fzsl[:, H : 2 * H]
                        else:
                            fsl = sb.tile([P, H], FP32, tag="fsl", bufs=ldb)
                            nc.sync.dma_start(out=fsl, in_=yr[:, j, b, 0:H])
                            zsl = sb.tile([P, H], FP32, tag="zsl", bufs=ldb)
                            z_eng = nc.gpsimd if z_swdge else nc.sync
                            z_eng.dma_start(out=zsl, in_=yr[:, j, b, H : 2 * H])
                        osl = sb.tile([P, H], FP32, tag="osl", bufs=ldb)
                        o_eng = nc.gpsimd if o_swdge else nc.sync
                        o_eng.dma_start(out=osl, in_=yr[:, j, b, 2 * H : 3 * H])

                        fb, zb = (1, 2) if psum_swap else (2, 1)
                        ps_f = psum.tile([P, H], FP32, tag="ps_f", bufs=1 if p3full else fb)
                        ps_z = psum.tile([P, H], FP32, tag="ps_z", bufs=zb)
                        for hb in range(HB):
                            nc.tensor.transpose(
                                ps_f[:, hb * P : (hb + 1) * P],
                                fsl[:, hb * P : (hb + 1) * P],
                                ident,
                            )
                        for hb in range(HB):
                            nc.tensor.transpose(
                                ps_z[:, hb * P : (hb + 1) * P],
                                zsl[:, hb * P : (hb + 1) * P],
                                ident,
                            )

                        # ACT drains PSUM: s_neg = 1 - sigmoid(F), zt = tanh(Z)
                        s_neg = sb.tile([P, H], FP32, tag="s_neg", bufs=3)
                        nc.scalar.activation(
                            s_neg, ps_f, mybir.ActivationFunctionType.Sigmoid,
                            scale=-1.0,
                        )
                        zt = sb.tile([P, H], FP32, tag="zt", bufs=3)
                        nc.scalar.activation(
                            zt, ps_z, mybir.ActivationFunctionType.Tanh
                        )
                        # sigmoid(O) early so phase 3 never waits on ACT
                        o_sig = sb.tile([P, H], FP32, tag="o_sig", bufs=ldb)
                        nc.scalar.activation(
                            o_sig, osl, mybir.ActivationFunctionType.Sigmoid
                        )

                        sr = s_neg.rearrange("p (hb t) -> p hb t", hb=HB)
                        zr = zt.rearrange("p (hb t) -> p hb t", hb=HB)
                        if scan_batch > 1:
                            f_t = f_tb[:, :, j, :]
                            zf = zfb[:, :, j, :]
                        else:
                            # per-j gate tensors [p, hb, t] — consumed by this
                            # j's scans immediately, no persistence needed
                            f_t = sb.tile([P, HB, P], FP32, tag="f_t", bufs=3)
                            zf = sb.tile([P, HB, P], FP32, tag="zf", bufs=3)
                        # f = 1 - s_neg on Pool; zf = s_neg * tanh(z) on DVE
                        nc.gpsimd.tensor_scalar(
                            f_t, sr, -1.0, 1.0,
                            op0=mybir.AluOpType.mult, op1=mybir.AluOpType.add,
                        )
                        if zf_split:
                            # split zf across DVE and Pool to relieve DVE
                            hh = HB // 2
                            nc.vector.tensor_mul(
                                zf[:, :hh, :], zr[:, :hh, :], sr[:, :hh, :]
                            )
                            nc.gpsimd.tensor_mul(
                                zf[:, hh:, :], zr[:, hh:, :], sr[:, hh:, :]
                            )
                        else:
                            nc.vector.tensor_mul(zf, zr, sr)

                        if scan_batch > 1:
                            # scans emitted at pair/batch boundaries below
                            pending.append((b, j, c_t, o_sig))
                            pairs = scan_mode == "pairs" or (
                                scan_mode == "hybrid" and b == BS - 1
                            )
                            if pairs and j % 2 == 1:
                                j0 = j - 1
                                for hb in range(HB):
                                    nc.vector.tensor_tensor_scan(
                                        c_t[:, hb, j0 : j + 1, :].rearrange(
                                            "p j t -> p (j t)"
                                        ),
                                        f_tb[:, hb, j0 : j + 1, :].rearrange(
                                            "p j t -> p (j t)"
                                        ),
                                        zfb[:, hb, j0 : j + 1, :].rearrange(
                                            "p j t -> p (j t)"
                                        ),
                                        initial=(
                                            ic_all[:, b, hb : hb + 1]
                                            if j0 == 0
                                            else c_t[:, hb, j0 - 1, P - 1 : P]
                                        ),
                                        op0=mybir.AluOpType.mult,
                                        op1=mybir.AluOpType.add,
                                    )
                                while pending:
                                    _phase3(*pending.pop(0))
                            continue

                        # phase 2: chained chunk scans — c for this t-chunk is
                        # ready as soon as this chunk's gates are, instead of
                        # waiting for the whole sequence
                        if scan_fake:
                            # timing diagnostic only: same shapes/deps, no scan
                            nc.vector.tensor_mul(
                                c_t[:, :, j, :],
                                f_t.rearrange("p hb t -> p (hb t)").rearrange(
                                    "p (hb t) -> p hb t", hb=HB
                                ),
                                zf,
                            )
                        else:
                            for hb in range(HB):
                                nc.vector.tensor_tensor_scan(
                                    c_t[:, hb, j, :],
                                    f_t[:, hb, :],
                                    zf[:, hb, :],
                                    initial=(
                                        ic_all[:, b, hb : hb + 1]
                                        if j == 0
                                        else c_t[:, hb, j - 1, P - 1 : P]
                                    ),
                                    op0=mybir.AluOpType.mult,
                                    op1=mybir.AluOpType.add,
                                )

                        pending.append((b, j, c_t, o_sig))
                        if len(pending) > defer:
                            _phase3(*pending.pop(0))

                    if scan_batch > 1 and not (
                        scan_mode == "pairs"
                        or (scan_mode == "hybrid" and b == BS - 1)
                    ):
                        # batched recurrence: one N=512 scan per (b, hb)
                        for hb in range(HB):
                            nc.vector.tensor_tensor_scan(
                                c_t[:, hb, :, :].rearrange("p j t -> p (j t)"),
                                f_tb[:, hb, :, :].rearrange("p j t -> p (j t)"),
                                zfb[:, hb, :, :].rearrange("p j t -> p (j t)"),
                                initial=ic_all[:, b, hb : hb + 1],
                                op0=mybir.AluOpType.mult,
                                op1=mybir.AluOpType.add,
                            )
                        while pending:
                            _phase3(*pending.pop(0))

                for args in pending:
                    _phase3(*args)

            if repeat == 1:
                _rep_body()
            else:
                # timing mode: hardware loop keeps the NEFF size constant in
                # `repeat`, so two loop bounds can be wall-clock diffed
                with tc.For_i(0, repeat, 1, staggered_reset=staggered):
                    _rep_body()
    nc.compile()
    return nc


def _get_nc() -> bass.Bass:
    if not _nc_cache:
        _nc_cache.append(_build_bass())
    return _nc_cache[0]


def kernel(Y: np.ndarray, init_c: np.ndarray, init_h: np.ndarray) -> np.ndarray:
    Y = np.ascontiguousarray(np.asarray(Y, dtype=np.float32))
    init_c = np.ascontiguousarray(np.asarray(init_c, dtype=np.float32))
    init_h = np.ascontiguousarray(np.asarray(init_h, dtype=np.float32))

    in_maps = []
    for k in range(N_CORES):
        sl = slice(k * BS, (k + 1) * BS)
        in_maps.append(
            {
                "Y": np.ascontiguousarray(Y[:, sl, :]),
                "init_c": np.ascontiguousarray(init_c[:, sl, :]),
                "init_h": np.ascontiguousarray(init_h[:, sl, :]),
            }
        )

    nc = _get_nc()
    res = run_bass_kernel_spmd(nc, in_maps, core_ids=list(range(N_CORES)))
    return np.concatenate([r["out"] for r in res.results], axis=1)
